# revision 13
# baseline (speedup 1.0000x reference)
"""Distributed Trainium2 kernel for fused multi-head attention
(QKV proj + RoPE + causal/key-padded SDPA + out-proj + bias).

Sharding: tensor-parallel over heads across 8 cores (2 heads/core, both
batches on every core).  After attention, one AllToAll per batch
converts head-shards into sequence-row-shards so the output projection
is computed locally per row slice; the host concatenates the 8 row
slices.

Schedule (PE executes in emission order, so emission order IS the
schedule): QKV for both batches, then attention pairs batch-grouped with
the shorter-L batch first (its pairs are cheaper, so the first AllToAll
issues as early as possible), then projection in the same batch order.
Collectives cost ~35 us nearly independent of size, so there are only
two, each overlapping the remaining attention/projection work.

Inside a pair, score tiles are processed in groups of two key tiles
(one 2-bank PSUM tile per group) with a one-group software pipeline:
the next group's QK matmuls are emitted before the previous group's
AV/denominator matmuls, so the exp activation always overlaps PE work.

Precision: QKV projection runs fp32r (full-rate fp32, moving dim >=
256).  q/k stay resident in SBUF as bf16; scores accumulate in fp32
PSUM; exp output, V, the AllToAll payload and the whole projection run
bf16 (fp32 PSUM accumulation).  Measured end-to-end relative error
~4e-3 against the fp32 reference.

Key padding is multiplicative instead of additive: V rows and the
denominator ones-vector are zeroed for tokens >= L, so the exp
activation needs no per-key-tile bias and batches over a whole group in
one instruction.  The causal mask is a post-exp affine_select zero
fill; columns below a diagonal tile's valid range hold stale PSUM that
the select also zeroes (matmul moving dims are kept >= 256, where
fp32r/bf16 run at full rate).

The kernel is compiled per (ceil(L/128), ceil(L/512)) signature: key
tiles and K/V projection chunks that are entirely masked (k >= L) are
skipped at compile time.  Any L value produces a correct kernel; the
compile cache is keyed on the derived bounds.
"""

import numpy as np
import ml_dtypes

import concourse.bacc as bacc
import concourse.bass as bass
import concourse.mybir as mybir
import concourse.tile as tile
from concourse import bass_utils

B, N, D, NH = 2, 2048, 2048, 16
HD = 128               # head dim
NCORES = 8
HL = NH // NCORES      # heads per core = 2
DL = HL * HD           # local model cols = 256
NS = N // NCORES       # output row slice per core = 256
HALF = HD // 2
ET = D // HD           # 16 contraction tiles
NT = N // HD           # 16 seq tiles of 128
NCH = N // 512         # 4 free-dim chunks of 512
GRP = 2                # key tiles per exp group (2 PSUM banks)
SCALE = 1.0 / float(np.sqrt(HD))
ROPE_BASE = 10000.0

F32 = mybir.dt.float32
F32R = mybir.dt.float32r
BF16 = mybir.dt.bfloat16
I32 = mybir.dt.int32

_CACHE = {}


def _bounds(L):
    """Per-batch compile-time loop bounds from the key-padding lengths."""
    jt = tuple(min(NT, max(1, -(-int(l) // HD))) for l in L)    # key tiles
    kvc = tuple(min(NCH, max(1, -(-int(l) // 512))) for l in L)  # k/v chunks
    return jt, kvc


def build(jtmax, kvcmax):
    key = (jtmax, kvcmax)
    if key in _CACHE:
        return _CACHE[key]
    # process the shorter batch first: its attention pairs are cheaper,
    # so its AllToAll issues earlier and hides under the longer batch
    border = sorted(range(B), key=lambda b: jtmax[b])

    nc = bacc.Bacc("TRN2", target_bir_lowering=False, debug=False,
                   num_devices=NCORES)
    xT = nc.dram_tensor("xT", [B, D, N], F32R, kind="ExternalInput")
    wqkvT = nc.dram_tensor("wqkvT", [3, D, DL], F32R, kind="ExternalInput")
    wpT = nc.dram_tensor("wpT", [D, D], BF16, kind="ExternalInput")
    bp = nc.dram_tensor("bp", [1, D], F32, kind="ExternalInput")
    cosT = nc.dram_tensor("cosT", [HD, N], F32, kind="ExternalInput")
    sinT = nc.dram_tensor("sinT", [HD, N], F32, kind="ExternalInput")
    Lw = nc.dram_tensor("Lw", [1, B], I32, kind="ExternalInput")
    out = nc.dram_tensor("out", [B, NS, D], F32, kind="ExternalOutput")

    AF = mybir.ActivationFunctionType
    ALU = mybir.AluOpType

    with tile.TileContext(nc) as tc:
        with tc.tile_pool(name="persist", bufs=1) as pp, \
             tc.tile_pool(name="dram", bufs=1, space="DRAM") as dp:
            # one AllToAll per batch, issued after both of its pairs
            ca = [dp.tile([NCORES, HL, HD, NS], BF16, name=f"ca{b}")
                  for b in range(B)]
            cb = [dp.tile([NCORES, HL, HD, NS], BF16, name=f"cb{b}")
                  for b in range(B)]

            # V stays resident as masked bf16 until the projection phase
            # (opened before the weight pool: releases must be LIFO)
            vsctx = tc.tile_pool(name="vres", bufs=1)
            vsp = vsctx.__enter__()
            vsb = [vsp.tile([HD, NT, DL], BF16, name=f"vsb{b}")
                   for b in range(B)]

            # first x chunk before the weight stream so the PE starts
            # within a few us; weights chunked per contraction tile so the
            # first matmul only waits for one 128-row slice of each
            xp_ctx = tc.tile_pool(name="ph1x", bufs=20)
            xp = xp_ctx.__enter__()
            xts0 = []
            for et in range(ET):
                xt = xp.tile([HD, 512], F32R, tag="xt", name=f"xt0{et}")
                nc.sync.dma_start(
                    xt[:], xT[border[0], et * HD:(et + 1) * HD, 0:512])
                xts0.append(xt)

            wqp_ctx = tc.tile_pool(name="wqkv", bufs=1)
            wqp = wqp_ctx.__enter__()
            wq = wqp.tile([HD, ET, DL], F32R, tag="wq")
            wk = wqp.tile([HD, ET, DL], F32R, tag="wk")
            wv = wqp.tile([HD, ET, DL], F32R, tag="wv")
            for et in range(ET):
                for w, i in ((wq, 0), (wk, 1), (wv, 2)):
                    nc.sync.dma_start(
                        w[:, et, :], wqkvT[i, et * HD:(et + 1) * HD, :])

            # rope tables duplicated across both partition halves
            cosb = pp.tile([HD, N], F32)
            nc.sync.dma_start(cosb[:], cosT[:])
            sinb = pp.tile([HD, N], F32)
            nc.sync.dma_start(sinb[:], sinT[:])

            # token-validity masks per (partition = tok%128, batch, tile):
            # f32 for masking V during the PSUM copy, bf16 as the
            # denominator matmul's stationary column
            iota = pp.tile([HD, NT], I32)
            nc.gpsimd.iota(iota[:], pattern=[[HD, NT]], base=0,
                           channel_multiplier=1)
            iotaf = pp.tile([HD, NT], F32)
            nc.vector.tensor_copy(iotaf[:], iota[:])
            lsb = pp.tile([1, B], I32)
            nc.sync.dma_start(lsb[:], Lw[:])
            lf = pp.tile([1, B], F32)
            nc.vector.tensor_copy(lf[:], lsb[:])
            lb = pp.tile([HD, B], F32)
            nc.gpsimd.partition_broadcast(lb[:], lf[:])
            mval = pp.tile([HD, B, NT], F32)
            mbf = pp.tile([HD, B, NT], BF16)
            for b in range(B):
                nc.vector.tensor_scalar(mval[:, b, :], iotaf[:],
                                        lb[:, b:b + 1], None, ALU.is_lt)
            nc.vector.tensor_copy(mbf[:], mval[:])

            # bias row broadcast to all 128 partitions
            bp1 = pp.tile([1, D], F32)
            nc.sync.dma_start(bp1[:], bp[:])
            bpb = pp.tile([HD, D], F32)
            nc.gpsimd.partition_broadcast(bpb[:], bp1[:])

            # q/k live in SBUF as bf16 for the whole kernel (no DRAM
            # roundtrip)
            qsb = [[pp.tile([HD, N], BF16, name=f"qsb{b}{h}")
                    for h in range(HL)] for b in range(B)]
            ksb = [[pp.tile([HD, N], BF16, name=f"ksb{b}{h}")
                    for h in range(HL)] for b in range(B)]

            # ---------------- Phase 1: QKV projection + RoPE ----------------
            with tc.tile_pool(name="ph1s", bufs=3) as sp, \
                 tc.tile_pool(name="ph1p", bufs=2, space="PSUM") as pq:
                for b in border:
                    for c4 in range(NCH):
                        kv = c4 < kvcmax[b]
                        nsl = slice(c4 * 512, (c4 + 1) * 512)
                        if b == border[0] and c4 == 0:
                            xts = xts0
                        else:
                            xts = []
                            for et in range(ET):
                                xt = xp.tile([HD, 512], F32R, tag="xt",
                                             name=f"xt{et}")
                                nc.sync.dma_start(
                                    xt[:], xT[b, et * HD:(et + 1) * HD, nsl])
                                xts.append(xt)
                        # two 4-bank sub-iterations (one per head) so the
                        # PSUM pool double-buffers and the PE never waits
                        # for the rope/copy epilogue
                        for h in range(HL):
                            psq = pq.tile([HD, 512], F32, tag="pq")
                            psk = pq.tile([HD, 512], F32, tag="pk",
                                          name="psk") if kv else None
                            psv = [pq.tile([HD, DL], F32, tag=f"pv{i}",
                                           name=f"psv{i}") for i in range(2)] \
                                if kv else None
                            for et in range(ET):
                                st = (et == 0)
                                en = (et == ET - 1)
                                nc.tensor.matmul(
                                    psq[:], wq[:, et, h * HD:(h + 1) * HD],
                                    xts[et][:], start=st, stop=en)
                                if not kv:
                                    continue
                                nc.tensor.matmul(
                                    psk[:], wk[:, et, h * HD:(h + 1) * HD],
                                    xts[et][:], start=st, stop=en)
                                for i in range(2):
                                    s4 = 2 * h + i
                                    nc.tensor.matmul(
                                        psv[i][:],
                                        xts[et][:, s4 * HD:(s4 + 1) * HD],
                                        wv[:, et, :], start=st, stop=en)
                            # copy-first RoPE: free the PSUM bank after one
                            # copy, rotate in SBUF (tables half-duplicated so
                            # every tensor_tensor has equal partition bases),
                            # write bf16 into the resident q/k buffers
                            pairs = [(psq, qsb[b][h])]
                            if kv:
                                pairs.append((psk, ksb[b][h]))
                            for src, dst in pairs:
                                stg = sp.tile([HD, 512], F32, tag="stg")
                                t12 = sp.tile([HD, 512], F32, tag="t12")
                                nc.vector.tensor_copy(stg[:], src[:])
                                nc.vector.tensor_mul(
                                    t12[:HALF, :], stg[HALF:, :],
                                    sinb[HALF:, nsl])
                                nc.vector.tensor_mul(
                                    t12[HALF:, :], stg[:HALF, :],
                                    sinb[:HALF, nsl])
                                nc.vector.tensor_mul(stg[:], stg[:],
                                                     cosb[:, nsl])
                                nc.vector.tensor_sub(dst[:HALF, nsl],
                                                     stg[:HALF, :],
                                                     t12[:HALF, :])
                                nc.vector.tensor_add(dst[HALF:, nsl],
                                                     stg[HALF:, :],
                                                     t12[HALF:, :])
                            if kv:
                                for i in range(2):
                                    tb = c4 * 4 + 2 * h + i
                                    nc.vector.tensor_scalar(
                                        vsb[b][:, tb, :], psv[i][:],
                                        mval[:, b, tb:tb + 1], None, ALU.mult)
            wqp_ctx.__exit__(None, None, None)
            xp_ctx.__exit__(None, None, None)

            # ------------- Phase 2: attention + one AllToAll per batch -----
            with tc.tile_pool(name="atts", bufs=4) as sp2, \
                 tc.tile_pool(name="attn", bufs=2) as sp3, \
                 tc.tile_pool(name="attp", bufs=2, space="PSUM") as pq2, \
                 tc.tile_pool(name="attpo", bufs=2, space="PSUM") as pq3:

                def qk_group(qt, kt, grp):
                    c4, jts = grp["c4"], grp["jts"]
                    pss = pq2.tile([HD, GRP * 512], F32, tag="pss",
                                   name="pss")
                    pt = sp2.tile([HD, GRP * 512], BF16, tag="pt", name="pt")
                    grp["pss"], grp["pt"] = pss, pt
                    for j, jt in enumerate(jts):
                        # diagonal tiles (jt = 4*c4+r, r>0) have no valid
                        # columns below 128*r; keep the matmul moving dim
                        # >= 256 (small-moving runs at quarter rate)
                        r = jt - 4 * c4
                        lo = min(128 * r, 256) if r > 0 else 0
                        grp["lo"][jt] = (j, lo, r)
                        nc.tensor.matmul(
                            pss[:, j * 512 + lo:(j + 1) * 512],
                            kt[:, jt * HD:(jt + 1) * HD],
                            qt[:, c4 * 512 + lo:(c4 + 1) * 512],
                            start=True, stop=True)
                    # one exp per group; columns below each tile's lo hold
                    # stale PSUM, exp'd then zeroed by the causal select
                    lo0 = grp["lo"][jts[0]][1]
                    gw = (grp["lo"][jts[-1]][0] + 1) * 512
                    nc.scalar.activation(pt[:, lo0:gw], pss[:, lo0:gw],
                                         AF.Exp, scale=SCALE)
                    for jt in jts:
                        j, lo, r = grp["lo"][jt]
                        if r >= 0:
                            nc.gpsimd.affine_select(
                                out=pt[:, j * 512 + lo:(j + 1) * 512],
                                in_=pt[:, j * 512 + lo:(j + 1) * 512],
                                compare_op=ALU.is_ge, fill=0.0,
                                base=lo - 128 * r,
                                pattern=[[1, 512 - lo]],
                                channel_multiplier=-1)

                def av_group(b, vt, grp, state):
                    c4, jts, njt = grp["c4"], grp["jts"], grp["njt"]
                    if jts[0] == 0:
                        state["pso"] = pq3.tile([HD, 512], F32, tag="pso",
                                                name="pso")
                        state["psd"] = pq3.tile([1, 512], F32, tag="psd",
                                                name="psd")
                    pso, psd = state["pso"], state["psd"]
                    pt = grp["pt"]
                    for jt in jts:
                        j, lo, r = grp["lo"][jt]
                        st = (jt == 0)
                        en = (jt == njt - 1)
                        nc.tensor.matmul(
                            pso[:, lo:], vt[:, jt, :],
                            pt[:, j * 512 + lo:(j + 1) * 512],
                            start=st, stop=en)
                        nc.tensor.matmul(
                            psd[:, lo:], mbf[:, b, jt:jt + 1],
                            pt[:, j * 512 + lo:(j + 1) * 512],
                            start=st, stop=en)
                    if jts[-1] == njt - 1:        # last group of this c4
                        rec = sp3.tile([1, 512], F32, tag="rec", name="rec")
                        nc.vector.reciprocal(rec[:], psd[:])
                        rb = sp3.tile([HD, 512], F32, tag="rb", name="rb")
                        nc.gpsimd.partition_broadcast(rb[:], rec[:])
                        ou = sp3.tile([HD, 512], BF16, tag="ou", name="ou")
                        nc.vector.tensor_mul(ou[:], pso[:], rb[:])
                        h = state["h"]
                        for r in range(2):
                            nc.sync.dma_start(
                                ca[b][2 * c4 + r, h],
                                ou[:, r * NS:(r + 1) * NS])

                for b in border:
                    for h in range(HL):
                        qt = qsb[b][h]
                        kt = ksb[b][h]
                        vt = vsb[b][:, :, h * HD:(h + 1) * HD]
                        grps = []
                        for c4 in range(NCH):
                            njt = min(4 * c4 + 4, jtmax[b])
                            for g in range(-(-njt // GRP)):
                                jts = list(range(
                                    GRP * g, min(GRP * g + GRP, njt)))
                                grps.append({"c4": c4, "jts": jts,
                                             "njt": njt, "lo": {}})
                        # two-group software pipeline: QK of groups i+1 and
                        # i+2 are emitted before AV of group i, so the
                        # exp -> causal-select chain (~2.3us) completes
                        # before AV needs the tile (pss frees at exp-read,
                        # so two PSUM bufs still suffice)
                        state = {"h": h}
                        qk_group(qt, kt, grps[0])
                        if len(grps) > 1:
                            qk_group(qt, kt, grps[1])
                        for i, grp in enumerate(grps):
                            if i + 2 < len(grps):
                                qk_group(qt, kt, grps[i + 2])
                            av_group(b, vt, grp, state)
                    # exchange this batch while the next batch computes
                    nc.gpsimd.collective_compute(
                        "AllToAll", mybir.AluOpType.bypass,
                        replica_groups=[list(range(NCORES))],
                        ins=[ca[b].opt()], outs=[cb[b].opt()])

            vsctx.__exit__(None, None, None)

            # ---------------- Phase 3: output projection ----------------
            # batch-major in the same order the exchanges complete
            with tc.tile_pool(name="proj", bufs=1) as pj, \
                 tc.tile_pool(name="projw", bufs=4) as pw, \
                 tc.tile_pool(name="projs", bufs=4) as po, \
                 tc.tile_pool(name="projp", bufs=4, space="PSUM") as pq4:
                # asb[h][p, b, s, n] = cb[b][s, h, p, n].  Per batch the
                # DMA order is wpt (dep-free prefetch) then asb (waits on
                # that batch's AllToAll) then compute, so a pending
                # collective never head-of-line-blocks the weight stream
                asb = [pj.tile([HD, B, NCORES, NS], BF16, name=f"asb{h}")
                       for h in range(HL)]
                for b in border:
                    wpts = []
                    for f4 in range(NCH):
                        fsl = slice(f4 * 512, (f4 + 1) * 512)
                        wpt = pw.tile([HD, ET, 512], BF16, tag="wpt",
                                      name="wpt")
                        nc.sync.dma_start(
                            wpt[:],
                            wpT[:, fsl].rearrange("(t p) f -> p t f", p=HD))
                        wpts.append(wpt)
                    for h in range(HL):
                        for s in range(NCORES):
                            nc.sync.dma_start(asb[h][:, b, s], cb[b][s, h])
                    for f4 in range(NCH):
                        fsl = slice(f4 * 512, (f4 + 1) * 512)
                        wpt = wpts[f4]
                        for m in range(NS // HD):
                            psp = pq4.tile([HD, 512], F32, tag="psp",
                                           name="psp")
                            for gi in range(ET):
                                h, s = gi // NCORES, gi % NCORES
                                nc.tensor.matmul(
                                    psp[:],
                                    asb[h][:, b, s, m * HD:(m + 1) * HD],
                                    wpt[:, 2 * s + h, :],
                                    start=(gi == 0), stop=(gi == ET - 1))
                            ot = po.tile([HD, 512], F32, tag="ot", name="ot")
                            nc.vector.tensor_add(ot[:], psp[:], bpb[:, fsl])
                            nc.sync.dma_start(
                                out[b, m * HD:(m + 1) * HD, fsl], ot[:])

    nc.compile()
    _CACHE[key] = nc
    return nc


def _prep_inputs(x, Wqkv, Wproj, bproj, L):
    x = np.asarray(x, np.float32)
    Wqkv = np.asarray(Wqkv, np.float32)
    Wproj = np.asarray(Wproj, np.float32)
    bproj = np.asarray(bproj, np.float32)
    L = np.asarray(L, np.int32)

    xT = np.ascontiguousarray(x.transpose(0, 2, 1))
    wpT = np.ascontiguousarray(Wproj.T.astype(ml_dtypes.bfloat16))
    inv = 1.0 / (ROPE_BASE ** (np.arange(0, HD, 2, dtype=np.float32) / HD))
    ang = np.arange(N, dtype=np.float32)[:, None] * inv[None, :]
    cos1 = np.cos(ang).T.astype(np.float32)          # [64, N]
    sin1 = np.sin(ang).T.astype(np.float32)
    cosT = np.ascontiguousarray(np.vstack([cos1, cos1]))   # [128, N]
    sinT = np.ascontiguousarray(np.vstack([sin1, sin1]))
    Lw = L.reshape(1, B).astype(np.int32)
    bp = bproj.reshape(1, D)

    in_maps = []
    for c in range(NCORES):
        sl = slice(c * DL, (c + 1) * DL)
        w3 = np.stack([
            np.ascontiguousarray(Wqkv[0 * D:1 * D][sl].T),
            np.ascontiguousarray(Wqkv[1 * D:2 * D][sl].T),
            np.ascontiguousarray(Wqkv[2 * D:3 * D][sl].T),
        ])
        in_maps.append({
            "xT": xT, "wqkvT": w3, "wpT": wpT, "bp": bp,
            "cosT": cosT, "sinT": sinT, "Lw": Lw,
        })
    return in_maps


def run(x, Wqkv, Wproj, bproj, L, trace=False, tmpdir=None):
    jtmax, kvcmax = _bounds(np.asarray(L).reshape(-1))
    nc = build(jtmax, kvcmax)
    in_maps = _prep_inputs(x, Wqkv, Wproj, bproj, L)
    kw = {}
    if tmpdir is not None:
        kw["tmpdir"] = tmpdir
    res = bass_utils.run_bass_kernel_spmd(
        nc, in_maps, core_ids=list(range(NCORES)), trace=trace, **kw)
    full = np.empty((B, N, D), np.float32)
    for c in range(NCORES):
        full[:, c * NS:(c + 1) * NS, :] = res.results[c]["out"]
    return full, res


def kernel(x, Wqkv, Wproj, bproj, L, n_heads):
    assert int(n_heads) == NH
    full, _ = run(x, Wqkv, Wproj, bproj, L, trace=False)
    return full


# revision 17
# speedup vs baseline: 1.0197x; 1.0197x over previous
"""Distributed Trainium2 kernel for fused multi-head attention
(QKV proj + RoPE + causal/key-padded SDPA + out-proj + bias).

Sharding: tensor-parallel over heads across 8 cores (2 heads/core, both
batches on every core).  After attention, one AllToAll per batch
converts head-shards into sequence-row-shards so the output projection
is computed locally per row slice; the host concatenates the 8 row
slices.

Schedule (PE executes in emission order, so emission order IS the
schedule): QKV for both batches, then attention pairs batch-grouped with
the shorter-L batch first (its pairs are cheaper, so the first AllToAll
issues as early as possible), then projection in the same batch order.
Collectives cost ~35 us nearly independent of size, so there are only
two, each overlapping the remaining attention/projection work.

Inside a pair, score tiles are processed in groups of two key tiles
(one 2-bank PSUM tile per group) with a one-group software pipeline:
the next group's QK matmuls are emitted before the previous group's
AV/denominator matmuls, so the exp activation always overlaps PE work.

Precision: bf16 operands everywhere with fp32 PSUM accumulation and a
fp32 softmax (scores accumulate in fp32, exp reads fp32).  Measured
end-to-end relative error ~5e-3 against the fp32 reference (tolerance
2e-2).

Key padding is multiplicative instead of additive: V rows and the
denominator ones-vector are zeroed for tokens >= L, so the exp
activation needs no per-key-tile bias and batches over a whole group in
one instruction.  The causal mask is a post-exp affine_select zero
fill; columns below a diagonal tile's valid range hold stale PSUM that
the select also zeroes (matmul moving dims are kept >= 256, where
fp32r/bf16 run at full rate).

The kernel is compiled per (ceil(L/128), ceil(L/512)) signature: key
tiles and K/V projection chunks that are entirely masked (k >= L) are
skipped at compile time.  Any L value produces a correct kernel; the
compile cache is keyed on the derived bounds.
"""

import numpy as np
import ml_dtypes

import concourse.bacc as bacc
import concourse.bass as bass
import concourse.mybir as mybir
import concourse.tile as tile
from concourse import bass_utils

B, N, D, NH = 2, 2048, 2048, 16
HD = 128               # head dim
NCORES = 8
HL = NH // NCORES      # heads per core = 2
DL = HL * HD           # local model cols = 256
NS = N // NCORES       # output row slice per core = 256
HALF = HD // 2
ET = D // HD           # 16 contraction tiles
NT = N // HD           # 16 seq tiles of 128
NCH = N // 512         # 4 free-dim chunks of 512
GRP = 2                # key tiles per exp group (2 PSUM banks)
SCALE = 1.0 / float(np.sqrt(HD))
ROPE_BASE = 10000.0

F32 = mybir.dt.float32
F32R = mybir.dt.float32r
BF16 = mybir.dt.bfloat16
I32 = mybir.dt.int32

_CACHE = {}


def _bounds(L):
    """Per-batch compile-time loop bounds from the key-padding lengths."""
    jt = tuple(min(NT, max(1, -(-int(l) // HD))) for l in L)    # key tiles
    kvc = tuple(min(NCH, max(1, -(-int(l) // 512))) for l in L)  # k/v chunks
    return jt, kvc


def build(jtmax, kvcmax):
    key = (jtmax, kvcmax)
    if key in _CACHE:
        return _CACHE[key]
    # process the shorter batch first: its attention pairs are cheaper,
    # so its AllToAll issues earlier and hides under the longer batch
    border = sorted(range(B), key=lambda b: jtmax[b])

    nc = bacc.Bacc("TRN2", target_bir_lowering=False, debug=False,
                   num_devices=NCORES)
    xT = nc.dram_tensor("xT", [B, D, N], BF16, kind="ExternalInput")
    wqkvT = nc.dram_tensor("wqkvT", [3, D, DL], BF16, kind="ExternalInput")
    wpT = nc.dram_tensor("wpT", [D, D], BF16, kind="ExternalInput")
    bp = nc.dram_tensor("bp", [1, D], F32, kind="ExternalInput")
    cosT = nc.dram_tensor("cosT", [HD, N], F32, kind="ExternalInput")
    sinT = nc.dram_tensor("sinT", [HD, N], F32, kind="ExternalInput")
    Lw = nc.dram_tensor("Lw", [1, B], I32, kind="ExternalInput")
    out = nc.dram_tensor("out", [B, NS, D], F32, kind="ExternalOutput")

    AF = mybir.ActivationFunctionType
    ALU = mybir.AluOpType

    with tile.TileContext(nc) as tc:
        with tc.tile_pool(name="persist", bufs=1) as pp, \
             tc.tile_pool(name="dram", bufs=1, space="DRAM") as dp:
            # one AllToAll per batch, issued after both of its pairs
            ca = [dp.tile([NCORES, HL, HD, NS], BF16, name=f"ca{b}")
                  for b in range(B)]
            cb = [dp.tile([NCORES, HL, HD, NS], BF16, name=f"cb{b}")
                  for b in range(B)]

            # projection weights pool opened first so its chunks can be
            # prefetched during attention (it is released last; pool
            # releases must be LIFO)
            pw_ctx = tc.tile_pool(name="projw", bufs=4)
            pw = pw_ctx.__enter__()

            # V stays resident as masked bf16 until the projection phase
            vsctx = tc.tile_pool(name="vres", bufs=1)
            vsp = vsctx.__enter__()
            vsb = [vsp.tile([HD, NT, DL], BF16, name=f"vsb{b}")
                   for b in range(B)]

            # q/k SBUF residency pool, released before the projection
            qk_ctx = tc.tile_pool(name="qkres", bufs=1)
            qkp = qk_ctx.__enter__()
            qsb = [[qkp.tile([HD, N], BF16, name=f"qsb{b}{h}")
                    for h in range(HL)] for b in range(B)]
            ksb = [[qkp.tile([HD, N], BF16, name=f"ksb{b}{h}")
                    for h in range(HL)] for b in range(B)]

            # first x chunk interleaved with the per-contraction-tile
            # weight chunks so the first matmul starts within ~2us and
            # each subsequent contraction tile unblocks incrementally
            xp_ctx = tc.tile_pool(name="ph1x", bufs=18)
            xp = xp_ctx.__enter__()
            wqp_ctx = tc.tile_pool(name="wqkv", bufs=1)
            wqp = wqp_ctx.__enter__()
            wq = wqp.tile([HD, ET, DL], BF16, tag="wq")
            wk = wqp.tile([HD, ET, DL], BF16, tag="wk")
            wv = wqp.tile([HD, ET, DL], BF16, tag="wv")
            xts0 = []
            for et in range(ET):
                xt = xp.tile([HD, 512], BF16, tag="xt", name=f"xt0{et}")
                nc.sync.dma_start(
                    xt[:], xT[border[0], et * HD:(et + 1) * HD, 0:512])
                xts0.append(xt)
                for w, i in ((wq, 0), (wk, 1), (wv, 2)):
                    nc.sync.dma_start(
                        w[:, et, :], wqkvT[i, et * HD:(et + 1) * HD, :])

            # rope tables duplicated across both partition halves
            cosb = pp.tile([HD, N], F32)
            nc.sync.dma_start(cosb[:], cosT[:])
            sinb = pp.tile([HD, N], F32)
            nc.sync.dma_start(sinb[:], sinT[:])

            # token-validity masks per (partition = tok%128, batch, tile):
            # f32 for masking V during the PSUM copy, bf16 as the
            # denominator matmul's stationary column
            iota = pp.tile([HD, NT], I32)
            nc.gpsimd.iota(iota[:], pattern=[[HD, NT]], base=0,
                           channel_multiplier=1)
            iotaf = pp.tile([HD, NT], F32)
            nc.vector.tensor_copy(iotaf[:], iota[:])
            lsb = pp.tile([1, B], I32)
            nc.sync.dma_start(lsb[:], Lw[:])
            lf = pp.tile([1, B], F32)
            nc.vector.tensor_copy(lf[:], lsb[:])
            lb = pp.tile([HD, B], F32)
            nc.gpsimd.partition_broadcast(lb[:], lf[:])
            mval = pp.tile([HD, B, NT], F32)
            mbf = pp.tile([HD, B, NT], BF16)
            for b in range(B):
                nc.vector.tensor_scalar(mval[:, b, :], iotaf[:],
                                        lb[:, b:b + 1], None, ALU.is_lt)
            nc.vector.tensor_copy(mbf[:], mval[:])

            # bias row broadcast to all 128 partitions
            bp1 = pp.tile([1, D], F32)
            nc.sync.dma_start(bp1[:], bp[:])
            bpb = pp.tile([HD, D], F32)
            nc.gpsimd.partition_broadcast(bpb[:], bp1[:])


            # ---------------- Phase 1: QKV projection + RoPE ----------------
            with tc.tile_pool(name="ph1s", bufs=3) as sp, \
                 tc.tile_pool(name="ph1p", bufs=2, space="PSUM") as pq:
                for b in border:
                    for c4 in range(NCH):
                        kv = c4 < kvcmax[b]
                        nsl = slice(c4 * 512, (c4 + 1) * 512)
                        if b == border[0] and c4 == 0:
                            xts = xts0
                        else:
                            xts = []
                            for et in range(ET):
                                xt = xp.tile([HD, 512], BF16, tag="xt",
                                             name=f"xt{et}")
                                nc.sync.dma_start(
                                    xt[:], xT[b, et * HD:(et + 1) * HD, nsl])
                                xts.append(xt)
                        # two 4-bank sub-iterations (one per head) so the
                        # PSUM pool double-buffers and the PE never waits
                        # for the rope/copy epilogue
                        for h in range(HL):
                            psq = pq.tile([HD, 512], F32, tag="pq")
                            psk = pq.tile([HD, 512], F32, tag="pk",
                                          name="psk") if kv else None
                            psv = [pq.tile([HD, DL], F32, tag=f"pv{i}",
                                           name=f"psv{i}") for i in range(2)] \
                                if kv else None
                            for et in range(ET):
                                st = (et == 0)
                                en = (et == ET - 1)
                                nc.tensor.matmul(
                                    psq[:], wq[:, et, h * HD:(h + 1) * HD],
                                    xts[et][:], start=st, stop=en)
                                if not kv:
                                    continue
                                nc.tensor.matmul(
                                    psk[:], wk[:, et, h * HD:(h + 1) * HD],
                                    xts[et][:], start=st, stop=en)
                                for i in range(2):
                                    s4 = 2 * h + i
                                    nc.tensor.matmul(
                                        psv[i][:],
                                        xts[et][:, s4 * HD:(s4 + 1) * HD],
                                        wv[:, et, :], start=st, stop=en)
                            # copy-first RoPE: free the PSUM bank after one
                            # copy, rotate in SBUF (tables half-duplicated so
                            # every tensor_tensor has equal partition bases),
                            # write bf16 into the resident q/k buffers
                            pairs = [(psq, qsb[b][h])]
                            if kv:
                                pairs.append((psk, ksb[b][h]))
                            for src, dst in pairs:
                                stg = sp.tile([HD, 512], F32, tag="stg")
                                t12 = sp.tile([HD, 512], F32, tag="t12")
                                nc.vector.tensor_copy(stg[:], src[:])
                                nc.vector.tensor_mul(
                                    t12[:HALF, :], stg[HALF:, :],
                                    sinb[HALF:, nsl])
                                nc.vector.tensor_mul(
                                    t12[HALF:, :], stg[:HALF, :],
                                    sinb[:HALF, nsl])
                                nc.vector.tensor_mul(stg[:], stg[:],
                                                     cosb[:, nsl])
                                nc.vector.tensor_sub(dst[:HALF, nsl],
                                                     stg[:HALF, :],
                                                     t12[:HALF, :])
                                nc.vector.tensor_add(dst[HALF:, nsl],
                                                     stg[HALF:, :],
                                                     t12[HALF:, :])
                            if kv:
                                for i in range(2):
                                    tb = c4 * 4 + 2 * h + i
                                    nc.vector.tensor_scalar(
                                        vsb[b][:, tb, :], psv[i][:],
                                        mval[:, b, tb:tb + 1], None, ALU.mult)
            wqp_ctx.__exit__(None, None, None)
            xp_ctx.__exit__(None, None, None)

            # ------------- Phase 2: attention + one AllToAll per batch -----
            with tc.tile_pool(name="atts", bufs=5) as sp2, \
                 tc.tile_pool(name="attn", bufs=2) as sp3, \
                 tc.tile_pool(name="attp", bufs=2, space="PSUM") as pq2, \
                 tc.tile_pool(name="attpo", bufs=2, space="PSUM") as pq3:

                def qk_group(qt, kt, grp):
                    c4, jts = grp["c4"], grp["jts"]
                    pss = pq2.tile([HD, GRP * 512], F32, tag="pss",
                                   name="pss")
                    pt = sp2.tile([HD, GRP * 512], BF16, tag="pt", name="pt")
                    grp["pss"], grp["pt"] = pss, pt
                    for j, jt in enumerate(jts):
                        # diagonal tiles (jt = 4*c4+r, r>0) have no valid
                        # columns below 128*r; keep the matmul moving dim
                        # >= 256 (small-moving runs at quarter rate)
                        r = jt - 4 * c4
                        lo = min(128 * r, 256) if r > 0 else 0
                        grp["lo"][jt] = (j, lo, r)
                        nc.tensor.matmul(
                            pss[:, j * 512 + lo:(j + 1) * 512],
                            kt[:, jt * HD:(jt + 1) * HD],
                            qt[:, c4 * 512 + lo:(c4 + 1) * 512],
                            start=True, stop=True)
                    # one exp per group; columns below each tile's lo hold
                    # stale PSUM, exp'd then zeroed by the causal select
                    lo0 = grp["lo"][jts[0]][1]
                    gw = (grp["lo"][jts[-1]][0] + 1) * 512
                    nc.scalar.activation(pt[:, lo0:gw], pss[:, lo0:gw],
                                         AF.Exp, scale=SCALE)
                    for jt in jts:
                        j, lo, r = grp["lo"][jt]
                        if r >= 0:
                            nc.gpsimd.affine_select(
                                out=pt[:, j * 512 + lo:(j + 1) * 512],
                                in_=pt[:, j * 512 + lo:(j + 1) * 512],
                                compare_op=ALU.is_ge, fill=0.0,
                                base=lo - 128 * r,
                                pattern=[[1, 512 - lo]],
                                channel_multiplier=-1)

                def av_group(b, vt, grp, state):
                    c4, jts, njt = grp["c4"], grp["jts"], grp["njt"]
                    if jts[0] == 0:
                        state["pso"] = pq3.tile([HD, 512], F32, tag="pso",
                                                name="pso")
                        state["psd"] = pq3.tile([1, 512], F32, tag="psd",
                                                name="psd")
                    pso, psd = state["pso"], state["psd"]
                    pt = grp["pt"]
                    for jt in jts:
                        j, lo, r = grp["lo"][jt]
                        st = (jt == 0)
                        en = (jt == njt - 1)
                        nc.tensor.matmul(
                            pso[:, lo:], vt[:, jt, :],
                            pt[:, j * 512 + lo:(j + 1) * 512],
                            start=st, stop=en)
                        nc.tensor.matmul(
                            psd[:, lo:], mbf[:, b, jt:jt + 1],
                            pt[:, j * 512 + lo:(j + 1) * 512],
                            start=st, stop=en)
                    if jts[-1] == njt - 1:        # last group of this c4
                        rec = sp3.tile([1, 512], F32, tag="rec", name="rec")
                        nc.vector.reciprocal(rec[:], psd[:])
                        rb = sp3.tile([HD, 512], F32, tag="rb", name="rb")
                        nc.gpsimd.partition_broadcast(rb[:], rec[:])
                        ou = sp3.tile([HD, 512], BF16, tag="ou", name="ou")
                        nc.vector.tensor_mul(ou[:], pso[:], rb[:])
                        h = state["h"]
                        for r in range(2):
                            nc.sync.dma_start(
                                ca[b][2 * c4 + r, h],
                                ou[:, r * NS:(r + 1) * NS])

                for b in border:
                    for h in range(HL):
                        qt = qsb[b][h]
                        kt = ksb[b][h]
                        vt = vsb[b][:, :, h * HD:(h + 1) * HD]
                        grps = []
                        for c4 in range(NCH):
                            njt = min(4 * c4 + 4, jtmax[b])
                            for g in range(-(-njt // GRP)):
                                jts = list(range(
                                    GRP * g, min(GRP * g + GRP, njt)))
                                grps.append({"c4": c4, "jts": jts,
                                             "njt": njt, "lo": {}})
                        # three-group software pipeline: the exp ->
                        # causal-select chain completes well before AV
                        # needs the tile, so the PE sequencer's run-ahead
                        # is never reset by a just-in-time wait and
                        # LDWEIGHTS overlaps the previous matmul (pss
                        # frees at exp-read, so two PSUM bufs suffice)
                        state = {"h": h}
                        for j0 in range(min(3, len(grps))):
                            qk_group(qt, kt, grps[j0])
                        for i, grp in enumerate(grps):
                            if i + 3 < len(grps):
                                qk_group(qt, kt, grps[i + 3])
                            av_group(b, vt, grp, state)
                    # exchange this batch while the next batch computes
                    nc.gpsimd.collective_compute(
                        "AllToAll", mybir.AluOpType.bypass,
                        replica_groups=[list(range(NCORES))],
                        ins=[ca[b].opt()], outs=[cb[b].opt()])
                    if b == border[0]:
                        # prefetch the first projection weight chunk while
                        # the second batch's attention runs
                        wpt = pw.tile([HD, ET, 512], BF16, tag="wpt",
                                      name="wpt")
                        nc.sync.dma_start(
                            wpt[:],
                            wpT[:, 0:512].rearrange("(t p) f -> p t f", p=HD))
                        wpt0 = wpt

            qk_ctx.__exit__(None, None, None)
            vsctx.__exit__(None, None, None)

            # ---------------- Phase 3: output projection ----------------
            # batch-major in the same order the exchanges complete
            with tc.tile_pool(name="proj", bufs=1) as pj, \
                 tc.tile_pool(name="projs", bufs=4) as po, \
                 tc.tile_pool(name="projp", bufs=4, space="PSUM") as pq4:
                # asb[h][p, b, s, n] = cb[b][s, h, p, n].  Per batch the
                # DMA order is wpt (dep-free prefetch) then asb (waits on
                # that batch's AllToAll) then compute, so a pending
                # collective never head-of-line-blocks the weight stream
                asb = [pj.tile([HD, B, NCORES, NS], BF16, name=f"asb{h}")
                       for h in range(HL)]
                for b in border:
                    wpts = []
                    for f4 in range(NCH):
                        if b == border[0] and f4 == 0:
                            wpts.append(wpt0)
                            continue
                        fsl = slice(f4 * 512, (f4 + 1) * 512)
                        wpt = pw.tile([HD, ET, 512], BF16, tag="wpt",
                                      name="wpt")
                        nc.sync.dma_start(
                            wpt[:],
                            wpT[:, fsl].rearrange("(t p) f -> p t f", p=HD))
                        wpts.append(wpt)
                    for h in range(HL):
                        for s in range(NCORES):
                            nc.sync.dma_start(asb[h][:, b, s], cb[b][s, h])
                    for f4 in range(NCH):
                        fsl = slice(f4 * 512, (f4 + 1) * 512)
                        wpt = wpts[f4]
                        for m in range(NS // HD):
                            psp = pq4.tile([HD, 512], F32, tag="psp",
                                           name="psp")
                            for gi in range(ET):
                                h, s = gi // NCORES, gi % NCORES
                                nc.tensor.matmul(
                                    psp[:],
                                    asb[h][:, b, s, m * HD:(m + 1) * HD],
                                    wpt[:, 2 * s + h, :],
                                    start=(gi == 0), stop=(gi == ET - 1))
                            ot = po.tile([HD, 512], F32, tag="ot", name="ot")
                            nc.vector.tensor_add(ot[:], psp[:], bpb[:, fsl])
                            nc.sync.dma_start(
                                out[b, m * HD:(m + 1) * HD, fsl], ot[:])

            pw_ctx.__exit__(None, None, None)

    nc.compile()
    _CACHE[key] = nc
    return nc


def _prep_inputs(x, Wqkv, Wproj, bproj, L):
    x = np.asarray(x, np.float32)
    Wqkv = np.asarray(Wqkv, np.float32)
    Wproj = np.asarray(Wproj, np.float32)
    bproj = np.asarray(bproj, np.float32)
    L = np.asarray(L, np.int32)

    xT = np.ascontiguousarray(
        x.transpose(0, 2, 1).astype(ml_dtypes.bfloat16))
    wpT = np.ascontiguousarray(Wproj.T.astype(ml_dtypes.bfloat16))
    inv = 1.0 / (ROPE_BASE ** (np.arange(0, HD, 2, dtype=np.float32) / HD))
    ang = np.arange(N, dtype=np.float32)[:, None] * inv[None, :]
    cos1 = np.cos(ang).T.astype(np.float32)          # [64, N]
    sin1 = np.sin(ang).T.astype(np.float32)
    cosT = np.ascontiguousarray(np.vstack([cos1, cos1]))   # [128, N]
    sinT = np.ascontiguousarray(np.vstack([sin1, sin1]))
    Lw = L.reshape(1, B).astype(np.int32)
    bp = bproj.reshape(1, D)

    in_maps = []
    for c in range(NCORES):
        sl = slice(c * DL, (c + 1) * DL)
        w3 = np.stack([
            np.ascontiguousarray(Wqkv[0 * D:1 * D][sl].T),
            np.ascontiguousarray(Wqkv[1 * D:2 * D][sl].T),
            np.ascontiguousarray(Wqkv[2 * D:3 * D][sl].T),
        ]).astype(ml_dtypes.bfloat16)
        in_maps.append({
            "xT": xT, "wqkvT": w3, "wpT": wpT, "bp": bp,
            "cosT": cosT, "sinT": sinT, "Lw": Lw,
        })
    return in_maps


def run(x, Wqkv, Wproj, bproj, L, trace=False, tmpdir=None):
    jtmax, kvcmax = _bounds(np.asarray(L).reshape(-1))
    nc = build(jtmax, kvcmax)
    in_maps = _prep_inputs(x, Wqkv, Wproj, bproj, L)
    kw = {}
    if tmpdir is not None:
        kw["tmpdir"] = tmpdir
    res = bass_utils.run_bass_kernel_spmd(
        nc, in_maps, core_ids=list(range(NCORES)), trace=trace, **kw)
    full = np.empty((B, N, D), np.float32)
    for c in range(NCORES):
        full[:, c * NS:(c + 1) * NS, :] = res.results[c]["out"]
    return full, res


def kernel(x, Wqkv, Wproj, bproj, L, n_heads):
    assert int(n_heads) == NH
    full, _ = run(x, Wqkv, Wproj, bproj, L, trace=False)
    return full
